# revision 16
# baseline (speedup 1.0000x reference)
"""Contrastive (InfoNCE-style) loss kernel for Trainium2, SPMD over 8 NeuronCores.

Math: emb [2, N, D] -> v1 = l2norm(emb[0]), v2 = l2norm(emb[1])
  loss = -sum_i [ (v1_i . v2_i)/T - log sum_j exp((v1_i . v2_j)/T) ]

Sharding: each core receives only its 2048-row shard of BOTH views (2MB).
The second view is normalized, transposed and bf16-cast on its owner core,
then all-gathered on-device (512KB/rank), so the host ships 16MB total
instead of 80MB and per-core w-prep is done once globally instead of 8x.

Each core computes its [2048, 16384] similarity block in 128x2048 PSUM
tiles (bf16 matmul, fp32 accumulate). The exp+row-sum over those tiles is
split across two engines running concurrently:
  - ScalarE (ACT): exact exp with fused row-sum (accum_out), u's
    normalization folded into the per-partition activation scale.
  - VectorE (DVE): Schraudolph-style exp - one tensor_scalar computes
    a*x+b and converts to int16, whose bits ARE the bf16 encoding of
    2^(x*log2e) (piecewise-linear mantissa); pairs of tiles are then
    summed with a fused row-reduce (tensor_tensor_reduce). The constant
    b is bias-fitted so the sawtooth error averages out over the 16k-term
    row sums; the log() in the loss compresses what remains below 1e-4.

Per-row partials (ttl, draw, |u|^2, |w|^2) return to the host, which
combines in f64.
"""

import math
from contextlib import ExitStack

import numpy as np

import concourse.bass as bass
import concourse.bacc as bacc
import concourse.mybir as mybir
from concourse.masks import make_identity
from concourse.tile import TileContext

P = 128
D = 128
TEMP = 0.2
N_TOTAL = 16384
N_CORES = 8
M_CORE = N_TOTAL // N_CORES   # 2048 rows per core
S_BLOCKS = M_CORE // P        # 16 row-blocks per core
GW = M_CORE                   # sim columns per group (= one shard)
MM_N = 512                    # moving-operand columns per matmul

# Schraudolph exp constants: int16(round(A*x + B)) bit-viewed as bf16
# approximates e^x.  B includes a bias fit (c=0.058) that zeroes the mean
# relative error over the actual similarity distribution.
A_EXP = 128.0 * 1.4426950408889634
B_EXP = 16256.0 - 128.0 * 0.058

# m-blocks whose exp runs on DVE (Schraudolph); the rest run on ACT.
DVE_MS = (1, 4, 6, 9, 11, 14)

f32 = mybir.dt.float32
bf16 = mybir.dt.bfloat16
i16 = mybir.dt.int16


def build_kernel(dve_ms=DVE_MS, use_ttr: bool = True) -> bass.Bass:
    DVE_MS = tuple(dve_ms)
    n_groups = N_CORES
    mult = mybir.AluOpType.mult
    add = mybir.AluOpType.add
    Ln = mybir.ActivationFunctionType.Ln
    Exp = mybir.ActivationFunctionType.Exp

    nc = bacc.Bacc(num_devices=N_CORES)
    u_in = nc.declare_dram_parameter("u", [M_CORE, D], f32, isOutput=False)
    wown_in = nc.declare_dram_parameter("wown", [M_CORE, D], f32, isOutput=False)
    ttl_out = nc.declare_dram_parameter("ttl", [P, S_BLOCKS], f32, isOutput=True)
    draw_out = nc.declare_dram_parameter("draw", [P, S_BLOCKS], f32, isOutput=True)
    nsqu_out = nc.declare_dram_parameter("nsqu", [P, S_BLOCKS], f32, isOutput=True)
    nsqw_out = nc.declare_dram_parameter("nsqw", [P, S_BLOCKS], f32, isOutput=True)

    # Partition p holds a contiguous slab of rows: u4[p, s*D+d] = u[p*S_BLOCKS+s, d]
    u_ap = u_in[:].rearrange("(p s) d -> p (s d)", p=P)
    wown_ap = wown_in[:].rearrange("(p s) d -> p (s d)", p=P)

    with TileContext(nc) as tc, ExitStack() as ctx:
        consts = ctx.enter_context(tc.tile_pool(name="consts", bufs=1))
        big = ctx.enter_context(tc.tile_pool(name="big", bufs=1))
        small = ctx.enter_context(tc.tile_pool(name="small", bufs=1))
        sqp = ctx.enter_context(tc.tile_pool(name="sqp", bufs=2))
        esp = ctx.enter_context(tc.tile_pool(name="esp", bufs=2))
        ep = ctx.enter_context(tc.tile_pool(name="ep", bufs=2))
        wtp = ctx.enter_context(tc.tile_pool(name="wtp", bufs=3))
        psum = ctx.enter_context(tc.tile_pool(name="psum", bufs=2, space="PSUM"))
        dram = ctx.enter_context(tc.tile_pool(name="dram", bufs=1, space="DRAM"))

        identity = consts.tile([P, P], f32)
        make_identity(nc, identity)
        identity_bf = consts.tile([P, P], bf16)
        nc.vector.tensor_copy(out=identity_bf, in_=identity)
        neg_ln_t = consts.tile([P, 1], f32)
        nc.vector.memset(neg_ln_t, -math.log(TEMP))

        # PE observes the gpsimd (identity) semaphore here, so later real
        # transposes carry a single sync wait (Matmult allows only one).
        warm = psum.tile([P, GW], f32, tag="S")
        nc.tensor.transpose(warm[:, :P], identity, identity)

        u4 = big.tile([P, M_CORE], f32)
        wown4 = big.tile([P, M_CORE], f32)
        wn = big.tile([P, M_CORE], f32)     # normalized own w shard
        u_t = big.tile([P, M_CORE], bf16)
        wtx = big.tile([P, GW], bf16)       # own w_t (d-major), staged for AG

        nsqu = small.tile([P, S_BLOCKS], f32)
        nsqw = small.tile([P, S_BLOCKS], f32)
        draw = small.tile([P, S_BLOCKS], f32)
        ru = small.tile([P, S_BLOCKS], f32)
        ruA = small.tile([P, S_BLOCKS], f32)
        lnt = small.tile([P, S_BLOCKS], f32)
        rw = small.tile([P, S_BLOCKS], f32)
        tacc = small.tile([P, S_BLOCKS * n_groups], f32)
        parts = small.tile([P, S_BLOCKS * 4], f32)
        ttl = small.tile([P, S_BLOCKS], f32)
        escr = big.tile([P, GW], bf16)      # TTR main output (discarded)

        cc_in = dram.tile([P, GW], bf16)
        cc_out = dram.tile([N_CORES * P, GW], bf16, addr_space="Shared")

        dma = nc.sync

        # ---------------- w (own shard) prep: normalize, transpose, AG ----
        dma.dma_start(out=wown4, in_=wown_ap)
        dma.dma_start(out=u4, in_=u_ap)
        for s in range(S_BLOCKS):
            blk = slice(s * D, (s + 1) * D)
            sq = sqp.tile([P, D], f32, tag="sq")
            nc.vector.tensor_mul(out=sq, in0=wown4[:, blk], in1=wown4[:, blk])
            nc.vector.reduce_sum(out=nsqw[:, s:s + 1], in_=sq,
                                 axis=mybir.AxisListType.X)
        # rw = 1/|w| = exp(-0.5*ln(nsqw))
        nc.scalar.activation(out=lnt, in_=nsqw, func=Ln)
        nc.scalar.activation(out=rw, in_=lnt, func=Exp, scale=-0.5)
        for s in range(S_BLOCKS):
            blk = slice(s * D, (s + 1) * D)
            nc.vector.tensor_scalar(out=wn[:, blk], in0=wown4[:, blk],
                                    scalar1=rw[:, s:s + 1], scalar2=None,
                                    op0=mult)
        pw = psum.tile([P, GW], f32, tag="S")
        for s in range(S_BLOCKS):
            nc.tensor.transpose(pw[:, s * D:(s + 1) * D], wn[:, s * D:(s + 1) * D],
                                identity)
        nc.vector.tensor_copy(out=wtx, in_=pw)
        dma.dma_start(out=cc_in, in_=wtx)
        nc.gpsimd.collective_compute(
            "AllGather", mybir.AluOpType.bypass,
            replica_groups=[list(range(N_CORES))],
            ins=[cc_in[:].opt()], outs=[cc_out[:].opt()])

        # ---------------- u prep (overlaps with AG) -----------------------
        for s in range(S_BLOCKS):
            blk = slice(s * D, (s + 1) * D)
            sq = sqp.tile([P, D], f32, tag="sq")
            nc.vector.tensor_mul(out=sq, in0=u4[:, blk], in1=u4[:, blk])
            nc.vector.reduce_sum(out=nsqu[:, s:s + 1], in_=sq,
                                 axis=mybir.AxisListType.X)
            sq = sqp.tile([P, D], f32, tag="sq")
            nc.vector.tensor_mul(out=sq, in0=u4[:, blk], in1=wown4[:, blk])
            nc.vector.reduce_sum(out=draw[:, s:s + 1], in_=sq,
                                 axis=mybir.AxisListType.X)
        # ru = 1/(T*|u|) = exp(-0.5*ln(nsqu) - ln(T))
        nc.scalar.activation(out=lnt, in_=nsqu, func=Ln)
        nc.scalar.activation(out=ru, in_=lnt, func=Exp, scale=-0.5, bias=neg_ln_t)
        nc.vector.tensor_scalar(out=ruA, in0=ru, scalar1=A_EXP, scalar2=None,
                                op0=mult)
        dma.dma_start(out=nsqu_out[:], in_=nsqu)
        dma.dma_start(out=nsqw_out[:], in_=nsqw)
        dma.dma_start(out=draw_out[:], in_=draw)

        # u_t: transpose u (f32 -> PSUM), copy back casting to bf16
        pst = psum.tile([P, GW], f32, tag="S")
        for s in range(S_BLOCKS):
            blk = slice(s * D, (s + 1) * D)
            nc.tensor.transpose(pst[:, blk], u4[:, blk], identity)
        nc.vector.tensor_copy(out=u_t, in_=pst)

        # ---------------- main loop --------------------------------------
        dve_pending: dict = {}
        for g in range(n_groups):
            wt = wtp.tile([P, GW], bf16, tag="wt")
            dma.dma_start(out=wt, in_=cc_out[g * P:(g + 1) * P, :])
            for m in range(S_BLOCKS):
                ps = psum.tile([P, GW], f32, tag="S")
                for k4 in range(GW // MM_N):
                    nsl = slice(k4 * MM_N, (k4 + 1) * MM_N)
                    nc.tensor.matmul(ps[:, nsl], u_t[:, m * D:(m + 1) * D],
                                     wt[:, nsl], start=True, stop=True)
                if m in DVE_MS:
                    e = ep.tile([P, GW], i16, tag=f"e{m}")
                    nc.vector.tensor_scalar(out=e, in0=ps,
                                            scalar1=ruA[:, m:m + 1],
                                            scalar2=B_EXP, op0=mult, op1=add)
                    if not use_ttr:
                        nc.vector.reduce_sum(
                            out=tacc[:, m * n_groups + g:m * n_groups + g + 1],
                            in_=e[:].bitcast(bf16), axis=mybir.AxisListType.X)
                    elif g % 2 == 1:
                        ea = dve_pending.pop(m)
                        nc.vector.scalar_tensor_tensor(
                            out=escr, in0=ea[:].bitcast(bf16), scalar=1.0,
                            in1=e[:].bitcast(bf16), op0=mult, op1=add,
                            accum_out=parts[:, m * 4 + g // 2:m * 4 + g // 2 + 1])
                    else:
                        dve_pending[m] = e
                else:
                    es = esp.tile([P, GW], bf16, tag="es")
                    nc.scalar.activation(
                        out=es, in_=ps, func=Exp, scale=ru[:, m:m + 1],
                        accum_out=tacc[:, m * n_groups + g:m * n_groups + g + 1])
                    # WAR-ordered after the ACT read: makes DVE the last
                    # accessor of the PSUM slot so the next matmul's slot wait
                    # merges with its other DVE deps into one sync wait.
                    nc.vector.memset(ps[:, :1], 0.0)

        for m in range(S_BLOCKS):
            if m in DVE_MS and use_ttr:
                nc.vector.reduce_sum(out=ttl[:, m:m + 1],
                                     in_=parts[:, m * 4:(m + 1) * 4],
                                     axis=mybir.AxisListType.X)
            else:
                nc.vector.reduce_sum(out=ttl[:, m:m + 1],
                                     in_=tacc[:, m * n_groups:(m + 1) * n_groups],
                                     axis=mybir.AxisListType.X)
        dma.dma_start(out=ttl_out[:], in_=ttl)

    nc.compile()
    return nc


_NC_CACHE: dict = {}


def _get_nc() -> bass.Bass:
    if "nc" not in _NC_CACHE:
        _NC_CACHE["nc"] = build_kernel()
    return _NC_CACHE["nc"]


def make_in_maps(emb: np.ndarray) -> list[dict]:
    in_maps = []
    for c in range(N_CORES):
        sl = slice(c * M_CORE, (c + 1) * M_CORE)
        in_maps.append({
            "u": np.ascontiguousarray(emb[0, sl]),
            "wown": np.ascontiguousarray(emb[1, sl]),
        })
    return in_maps


def _combine(results: list[dict], temp: float = TEMP) -> np.float32:
    total = 0.0
    for r in results:
        ttl = r["ttl"].astype(np.float64).reshape(-1)
        draw = r["draw"].astype(np.float64).reshape(-1)
        nsqu = r["nsqu"].astype(np.float64).reshape(-1)
        nsqw = r["nsqw"].astype(np.float64).reshape(-1)
        norms = np.maximum(np.sqrt(nsqu), 1e-12) * np.maximum(np.sqrt(nsqw), 1e-12)
        total += np.sum(np.log(ttl) - draw / (temp * norms))
    return np.float32(total)


def kernel(emb: np.ndarray) -> np.ndarray:
    from concourse.bass_utils import run_bass_kernel_spmd

    emb = np.ascontiguousarray(np.asarray(emb, dtype=np.float32))
    assert emb.shape == (2, N_TOTAL, D), emb.shape
    nc = _get_nc()
    res = run_bass_kernel_spmd(nc, make_in_maps(emb), core_ids=list(range(N_CORES)))
    return np.array(_combine(res.results), dtype=np.float32)


# revision 36
# speedup vs baseline: 1.4387x; 1.4387x over previous
"""Contrastive (InfoNCE-style) loss kernel for Trainium2, SPMD over 8 NeuronCores.

Math: emb [2, N, D] -> v1 = l2norm(emb[0]), v2 = l2norm(emb[1])
  loss = -sum_i [ (v1_i . v2_i)/T - log sum_j exp((v1_i . v2_j)/T) ]

Sharding: each core receives only its 2048-row shard of BOTH views (2MB),
so the host ships 16MB total instead of 80MB. The second view is
normalized, transposed (d-major) and fp8e4-cast on its owner core, then
all-gathered on-device (512KB/rank) - w-prep is done once globally
instead of 8x per core.

Groups are processed in rotated order: group t covers shard (pid+t) mod 8
via runtime-offset DMA (bass.ds on the partition id), so group 0 is the
core's own shard, whose w_t is already in SBUF with no AllGather
dependency. That hides most of the collective's ~70us latency (and the
inter-core NEFF start skew it absorbs) behind real compute.

Each core computes its [2048, 16384] similarity block in 128x1024 PSUM
half-tiles (fp8 matmul, fp32 accumulate; 4 PSUM buffers give the PE a
deep enough pipeline window to stream at its instruction-rate floor).
The exp+row-sum over the tiles is split across two engines running
concurrently:
  - ScalarE (ACT): exact exp with fused row-sum (accum_out), u's
    normalization folded into the per-partition activation scale. The
    u/w squared-norms also run here (Square is in the same ACT table
    set as Ln/Exp, so there are no table switches).
  - VectorE (DVE): Schraudolph-style exp - one tensor_scalar computes
    a*x+b and converts to int16, whose bits ARE the bf16 encoding of
    2^(x*log2e) (piecewise-linear mantissa); pairs of tiles are then
    summed with a fused row-reduce (scalar_tensor_tensor accum_out).
    The constant b is bias-fitted so the sawtooth error averages out
    over the 16k-term row sums; the log() in the loss compresses what
    remains below 1e-4.

Per-row partials (ttl, draw, |u|^2, |w|^2) return to the host, which
combines in f64.
"""

import math
from contextlib import ExitStack

import numpy as np

import concourse.bass as bass
import concourse.bacc as bacc
import concourse.mybir as mybir
from concourse.masks import make_identity
from concourse.tile import TileContext

P = 128
D = 128
TEMP = 0.2
N_TOTAL = 16384
N_CORES = 8
M_CORE = N_TOTAL // N_CORES   # 2048 rows per core
S_BLOCKS = M_CORE // P        # 16 row-blocks per core
GW = M_CORE                   # sim columns per group (= one shard)
MM_N = 512                    # moving-operand columns per matmul

# Schraudolph exp constants: int16(round(A*x + B)) bit-viewed as bf16
# approximates e^x.  B includes a bias fit (c=0.058) that zeroes the mean
# relative error over the actual similarity distribution.
A_EXP = 128.0 * 1.4426950408889634
B_EXP = 16256.0 - 128.0 * 0.058

# m-blocks whose exp runs on DVE (Schraudolph); the rest run on ACT.
DVE_MS = (1, 4, 6, 9, 11, 14)

f32 = mybir.dt.float32
bf16 = mybir.dt.bfloat16
fp8 = mybir.dt.float8e4
i16 = mybir.dt.int16


def build_kernel(dve_ms=DVE_MS, use_ttr: bool = True, mm_n: int = MM_N,
                 act_memset: bool = False, half_psum: bool = True) -> bass.Bass:
    DVE_MS = tuple(dve_ms)
    MM_NL = mm_n
    n_groups = N_CORES
    mult = mybir.AluOpType.mult
    add = mybir.AluOpType.add
    Ln = mybir.ActivationFunctionType.Ln
    Exp = mybir.ActivationFunctionType.Exp

    nc = bacc.Bacc(num_devices=N_CORES)
    u_in = nc.declare_dram_parameter("u", [M_CORE, D], f32, isOutput=False)
    wown_in = nc.declare_dram_parameter("wown", [M_CORE, D], f32, isOutput=False)
    ttl_out = nc.declare_dram_parameter("ttl", [P, S_BLOCKS], f32, isOutput=True)
    draw_out = nc.declare_dram_parameter("draw", [P, S_BLOCKS], f32, isOutput=True)
    nsqu_out = nc.declare_dram_parameter("nsqu", [P, S_BLOCKS], f32, isOutput=True)
    nsqw_out = nc.declare_dram_parameter("nsqw", [P, S_BLOCKS], f32, isOutput=True)

    # Partition p holds a contiguous slab of rows: u4[p, s*D+d] = u[p*S_BLOCKS+s, d]
    u_ap = u_in[:].rearrange("(p s) d -> p (s d)", p=P)
    wown_ap = wown_in[:].rearrange("(p s) d -> p (s d)", p=P)

    with TileContext(nc) as tc, ExitStack() as ctx:
        consts = ctx.enter_context(tc.tile_pool(name="consts", bufs=1))
        big = ctx.enter_context(tc.tile_pool(name="big", bufs=1))
        small = ctx.enter_context(tc.tile_pool(name="small", bufs=1))
        sqp = ctx.enter_context(tc.tile_pool(name="sqp", bufs=2))
        esp = ctx.enter_context(tc.tile_pool(name="esp", bufs=2))
        ep = ctx.enter_context(tc.tile_pool(name="ep", bufs=2))
        wtp = ctx.enter_context(tc.tile_pool(name="wtp", bufs=3))
        psum = ctx.enter_context(tc.tile_pool(
            name="psum", bufs=4 if half_psum else 2, space="PSUM"))
        dram = ctx.enter_context(tc.tile_pool(name="dram", bufs=1, space="DRAM"))

        identity = consts.tile([P, P], f32)
        make_identity(nc, identity)
        neg_ln_t = consts.tile([P, 1], f32)
        nc.vector.memset(neg_ln_t, -math.log(TEMP))

        # PE observes the gpsimd (identity) semaphore here, so later real
        # transposes carry a single sync wait (Matmult allows only one).
        PSW = GW // 2 if half_psum else GW
        warm = psum.tile([P, PSW], f32, tag="S")
        nc.tensor.transpose(warm[:, :P], identity, identity)

        u4 = big.tile([P, M_CORE], f32)
        wown4 = big.tile([P, M_CORE], f32)
        wn = big.tile([P, M_CORE], f32)     # normalized own w shard
        u_t = big.tile([P, M_CORE], fp8)
        wtx = big.tile([P, GW], fp8)        # own w_t (d-major), staged for AG

        nsqu = small.tile([P, S_BLOCKS], f32)
        nsqw = small.tile([P, S_BLOCKS], f32)
        draw = small.tile([P, S_BLOCKS], f32)
        ru = small.tile([P, S_BLOCKS], f32)
        ruA = small.tile([P, S_BLOCKS], f32)
        lnt = small.tile([P, S_BLOCKS], f32)
        rw = small.tile([P, S_BLOCKS], f32)
        tacc = small.tile([P, S_BLOCKS * n_groups * 2], f32)
        nc.vector.memset(tacc, 0.0)
        parts = small.tile([P, S_BLOCKS * 4], f32)
        ttl = small.tile([P, S_BLOCKS], f32)
        escr = big.tile([P, GW], bf16)      # TTR main output (discarded)

        cc_in = dram.tile([P, GW], fp8)
        cc_out = dram.tile([N_CORES * P, GW], fp8, addr_space="Shared")

        dma = nc.sync

        # ---------------- w (own shard) prep: normalize, transpose, AG ----
        dma.dma_start(out=wown4, in_=wown_ap)
        dma.dma_start(out=u4, in_=u_ap)
        Square = mybir.ActivationFunctionType.Square
        for s in range(S_BLOCKS):
            blk = slice(s * D, (s + 1) * D)
            sq = sqp.tile([P, D], f32, tag="sq")
            nc.scalar.activation(out=sq, in_=wown4[:, blk], func=Square,
                                 accum_out=nsqw[:, s:s + 1])
        # rw = 1/|w| = exp(-0.5*ln(nsqw))
        nc.scalar.activation(out=lnt, in_=nsqw, func=Ln)
        nc.scalar.activation(out=rw, in_=lnt, func=Exp, scale=-0.5)
        for s in range(S_BLOCKS):
            blk = slice(s * D, (s + 1) * D)
            nc.vector.tensor_scalar(out=wn[:, blk], in0=wown4[:, blk],
                                    scalar1=rw[:, s:s + 1], scalar2=None,
                                    op0=mult)
        # u transposes go first on PE (they only need u4); the wn transposes
        # follow as soon as wn is scaled. DVE drains wtx (gating the AG)
        # before u_t (gating the first matmul).
        psts = []
        for h in range(GW // PSW):
            pst = psum.tile([P, PSW], f32, tag="S")
            for s in range(PSW // D):
                so = h * (PSW // D) + s
                nc.tensor.transpose(pst[:, s * D:(s + 1) * D],
                                    u4[:, so * D:(so + 1) * D], identity)
            psts.append(pst)
        pws = []
        for h in range(GW // PSW):
            pw = psum.tile([P, PSW], f32, tag="S")
            for s in range(PSW // D):
                so = h * (PSW // D) + s
                nc.tensor.transpose(pw[:, s * D:(s + 1) * D],
                                    wn[:, so * D:(so + 1) * D], identity)
            pws.append(pw)
        for h, pw in enumerate(pws):
            nc.vector.tensor_copy(out=wtx[:, h * PSW:(h + 1) * PSW], in_=pw)
        dma.dma_start(out=cc_in, in_=wtx)
        nc.gpsimd.collective_compute(
            "AllGather", mybir.AluOpType.bypass,
            replica_groups=[list(range(N_CORES))],
            ins=[cc_in[:].opt()], outs=[cc_out[:].opt()])

        # ---------------- u prep (overlaps with AG) -----------------------
        for s in range(S_BLOCKS):
            blk = slice(s * D, (s + 1) * D)
            sq = sqp.tile([P, D], f32, tag="sq")
            nc.scalar.activation(out=sq, in_=u4[:, blk], func=Square,
                                 accum_out=nsqu[:, s:s + 1])
        for h, pst in enumerate(psts):
            nc.vector.tensor_copy(out=u_t[:, h * PSW:(h + 1) * PSW], in_=pst)
        # ru = 1/(T*|u|) = exp(-0.5*ln(nsqu) - ln(T))
        nc.scalar.activation(out=lnt, in_=nsqu, func=Ln)
        nc.scalar.activation(out=ru, in_=lnt, func=Exp, scale=-0.5, bias=neg_ln_t)
        nc.vector.tensor_scalar(out=ruA, in0=ru, scalar1=A_EXP, scalar2=None,
                                op0=mult)
        for s in range(S_BLOCKS):
            blk = slice(s * D, (s + 1) * D)
            sq = sqp.tile([P, D], f32, tag="sq")
            nc.vector.tensor_mul(out=sq, in0=u4[:, blk], in1=wown4[:, blk])
            nc.vector.reduce_sum(out=draw[:, s:s + 1], in_=sq,
                                 axis=mybir.AxisListType.X)
        dma.dma_start(out=nsqu_out[:], in_=nsqu)
        dma.dma_start(out=nsqw_out[:], in_=nsqw)
        dma.dma_start(out=draw_out[:], in_=draw)

        # ---------------- main loop --------------------------------------
        # Rotated group order: group t covers shard (pid+t) mod 8, so group 0
        # is this core's own shard -- its w_t (wtx) is already in SBUF and
        # carries no AllGather dependency. This hides the AG's latency (and
        # the inter-core NEFF start skew it absorbs) behind real compute.
        pid = dma.partition_id()
        dve_pending: dict = {}
        for g in range(n_groups):
            if g == 0:
                wt = wtx
            else:
                wt = wtp.tile([P, GW], fp8, tag="wt")
                blk_idx = (pid | g) - (pid & g)
                dma.dma_start(out=wt, in_=cc_out[bass.ds(blk_idx * P, P), :])
            for m in range(S_BLOCKS):
                e = None
                for h in range(GW // PSW):
                    ps = psum.tile([P, PSW], f32, tag="S")
                    for k4 in range(PSW // MM_NL):
                        nsl = slice(k4 * MM_NL, (k4 + 1) * MM_NL)
                        wsl = slice(h * PSW + k4 * MM_NL,
                                    h * PSW + (k4 + 1) * MM_NL)
                        nc.tensor.matmul(ps[:, nsl], u_t[:, m * D:(m + 1) * D],
                                         wt[:, wsl], start=True, stop=True)
                    if m in DVE_MS:
                        if e is None:
                            e = ep.tile([P, GW], i16, tag=f"e{m}")
                        nc.vector.tensor_scalar(out=e[:, h * PSW:(h + 1) * PSW],
                                                in0=ps,
                                                scalar1=ruA[:, m:m + 1],
                                                scalar2=B_EXP, op0=mult, op1=add)
                    else:
                        es = esp.tile([P, PSW], bf16, tag="es")
                        nc.scalar.activation(
                            out=es, in_=ps, func=Exp, scale=ru[:, m:m + 1],
                            accum_out=tacc[:, (m * n_groups + g) * 2 + h:
                                           (m * n_groups + g) * 2 + h + 1])
                        if act_memset:
                            nc.vector.memset(ps[:, :1], 0.0)
                if m in DVE_MS:
                    if not use_ttr:
                        nc.vector.reduce_sum(
                            out=tacc[:, (m * n_groups + g) * 2:
                                     (m * n_groups + g) * 2 + 1],
                            in_=e[:].bitcast(bf16), axis=mybir.AxisListType.X)
                    elif g % 2 == 1:
                        ea = dve_pending.pop(m)
                        nc.vector.scalar_tensor_tensor(
                            out=escr, in0=ea[:].bitcast(bf16), scalar=1.0,
                            in1=e[:].bitcast(bf16), op0=mult, op1=add,
                            accum_out=parts[:, m * 4 + g // 2:m * 4 + g // 2 + 1])
                    else:
                        dve_pending[m] = e
                if g == n_groups - 1:
                    if m in DVE_MS and use_ttr:
                        nc.vector.reduce_sum(out=ttl[:, m:m + 1],
                                             in_=parts[:, m * 4:(m + 1) * 4],
                                             axis=mybir.AxisListType.X)
                    elif m in DVE_MS:
                        nc.vector.reduce_sum(
                            out=ttl[:, m:m + 1],
                            in_=tacc[:, m * n_groups * 2:(m + 1) * n_groups * 2:2],
                            axis=mybir.AxisListType.X)
                    else:
                        nc.vector.reduce_sum(
                            out=ttl[:, m:m + 1],
                            in_=tacc[:, m * n_groups * 2:(m + 1) * n_groups * 2],
                            axis=mybir.AxisListType.X)

        dma.dma_start(out=ttl_out[:], in_=ttl)

    nc.compile()
    return nc


_NC_CACHE: dict = {}


def _get_nc() -> bass.Bass:
    if "nc" not in _NC_CACHE:
        _NC_CACHE["nc"] = build_kernel()
    return _NC_CACHE["nc"]


def make_in_maps(emb: np.ndarray) -> list[dict]:
    in_maps = []
    for c in range(N_CORES):
        sl = slice(c * M_CORE, (c + 1) * M_CORE)
        in_maps.append({
            "u": np.ascontiguousarray(emb[0, sl]),
            "wown": np.ascontiguousarray(emb[1, sl]),
        })
    return in_maps


def _combine(results: list[dict], temp: float = TEMP) -> np.float32:
    total = 0.0
    for r in results:
        ttl = r["ttl"].astype(np.float64).reshape(-1)
        draw = r["draw"].astype(np.float64).reshape(-1)
        nsqu = r["nsqu"].astype(np.float64).reshape(-1)
        nsqw = r["nsqw"].astype(np.float64).reshape(-1)
        norms = np.maximum(np.sqrt(nsqu), 1e-12) * np.maximum(np.sqrt(nsqw), 1e-12)
        total += np.sum(np.log(ttl) - draw / (temp * norms))
    return np.float32(total)


def kernel(emb: np.ndarray) -> np.ndarray:
    from concourse.bass_utils import run_bass_kernel_spmd

    emb = np.ascontiguousarray(np.asarray(emb, dtype=np.float32))
    assert emb.shape == (2, N_TOTAL, D), emb.shape
    nc = _get_nc()
    res = run_bass_kernel_spmd(nc, make_in_maps(emb), core_ids=list(range(N_CORES)))
    return np.array(_combine(res.results), dtype=np.float32)


# revision 40
# speedup vs baseline: 1.6185x; 1.1250x over previous
"""Contrastive (InfoNCE-style) loss kernel for Trainium2, SPMD over 8 NeuronCores.

Math: emb [2, N, D] -> v1 = l2norm(emb[0]), v2 = l2norm(emb[1])
  loss = -sum_i [ (v1_i . v2_i)/T - log sum_j exp((v1_i . v2_j)/T) ]

Sharding: each core receives only its 2048-row shard of BOTH views (2MB),
so the host ships 16MB total instead of 80MB. The second view is
normalized, transposed (d-major) and fp8e4-cast on its owner core, then
all-gathered on-device (512KB/rank) - w-prep is done once globally
instead of 8x per core.

Groups are processed in rotated order: group t covers shard (pid+t) mod 8
via runtime-offset DMA (bass.ds on the partition id), so group 0 is the
core's own shard, whose w_t is already in SBUF with no AllGather
dependency. That hides most of the collective's ~70us latency (and the
inter-core NEFF start skew it absorbs) behind real compute.

Each core computes its [2048, 16384] similarity block in 128x1024 PSUM
half-tiles (fp8 matmul, fp32 accumulate; 4 PSUM buffers give the PE a
deep enough pipeline window to stream at its instruction-rate floor).
The exp+row-sum over the tiles is split across two engines running
concurrently:
  - ScalarE (ACT): exact exp with fused row-sum (accum_out), u's
    normalization folded into the per-partition activation scale. The
    u/w squared-norms also run here (Square is in the same ACT table
    set as Ln/Exp, so there are no table switches).
  - VectorE (DVE): Schraudolph-style exp - one tensor_scalar computes
    a*x+b and converts to int16, whose bits ARE the bf16 encoding of
    2^(x*log2e) (piecewise-linear mantissa); pairs of tiles are then
    summed with a fused row-reduce (scalar_tensor_tensor accum_out).
    The constant b is bias-fitted so the sawtooth error averages out
    over the 16k-term row sums; the log() in the loss compresses what
    remains below 1e-4.

Per-row partials (ttl, draw, |u|^2, |w|^2) return to the host, which
combines in f64.
"""

import math
from contextlib import ExitStack

import numpy as np

import concourse.bass as bass
import concourse.bacc as bacc
import concourse.mybir as mybir
from concourse.masks import make_identity
from concourse.tile import TileContext

P = 128
D = 128
TEMP = 0.2
N_TOTAL = 16384
N_CORES = 8
M_CORE = N_TOTAL // N_CORES   # 2048 rows per core
S_BLOCKS = M_CORE // P        # 16 row-blocks per core
GW = M_CORE                   # sim columns per group (= one shard)
MM_N = 512                    # moving-operand columns per matmul

# Schraudolph exp constants: int16(round(A*x + B)) bit-viewed as bf16
# approximates e^x.  B includes a bias fit (c=0.058) that zeroes the mean
# relative error over the actual similarity distribution.
A_EXP = 128.0 * 1.4426950408889634
B_EXP = 16256.0 - 128.0 * 0.058

# m-blocks whose exp runs on DVE (Schraudolph); the rest run on ACT.
DVE_MS = (1, 4, 6, 9, 11, 14)

f32 = mybir.dt.float32
bf16 = mybir.dt.bfloat16
fp8 = mybir.dt.float8e4
i16 = mybir.dt.int16


def build_kernel(dve_ms=DVE_MS, mm_n: int = MM_N,
                 act_memset: bool = False, half_psum: bool = True,
                 gp_stt: bool = False, own_act: bool = True) -> bass.Bass:
    DVE_MS = tuple(dve_ms)
    MM_NL = mm_n
    n_groups = N_CORES
    mult = mybir.AluOpType.mult
    add = mybir.AluOpType.add
    Ln = mybir.ActivationFunctionType.Ln
    Exp = mybir.ActivationFunctionType.Exp

    nc = bacc.Bacc(num_devices=N_CORES)
    u_in = nc.declare_dram_parameter("u", [M_CORE, D], f32, isOutput=False)
    wown_in = nc.declare_dram_parameter("wown", [M_CORE, D], f32, isOutput=False)
    ttl_out = nc.declare_dram_parameter("ttl", [P, S_BLOCKS], f32, isOutput=True)
    draw_out = nc.declare_dram_parameter("draw", [P, S_BLOCKS], f32, isOutput=True)
    nsqu_out = nc.declare_dram_parameter("nsqu", [P, S_BLOCKS], f32, isOutput=True)
    nsqw_out = nc.declare_dram_parameter("nsqw", [P, S_BLOCKS], f32, isOutput=True)

    # Partition p holds a contiguous slab of rows: u4[p, s*D+d] = u[p*S_BLOCKS+s, d]
    u_ap = u_in[:].rearrange("(p s) d -> p (s d)", p=P)
    wown_ap = wown_in[:].rearrange("(p s) d -> p (s d)", p=P)

    with TileContext(nc) as tc, ExitStack() as ctx:
        consts = ctx.enter_context(tc.tile_pool(name="consts", bufs=1))
        big = ctx.enter_context(tc.tile_pool(name="big", bufs=1))
        small = ctx.enter_context(tc.tile_pool(name="small", bufs=1))
        sqp = ctx.enter_context(tc.tile_pool(name="sqp", bufs=2))
        esp = ctx.enter_context(tc.tile_pool(name="esp", bufs=2))
        ep = ctx.enter_context(tc.tile_pool(name="ep", bufs=2))
        wtp = ctx.enter_context(tc.tile_pool(name="wtp", bufs=3))
        psum = ctx.enter_context(tc.tile_pool(
            name="psum", bufs=4 if half_psum else 2, space="PSUM"))
        dram = ctx.enter_context(tc.tile_pool(name="dram", bufs=1, space="DRAM"))

        identity = consts.tile([P, P], f32)
        make_identity(nc, identity)
        neg_ln_t = consts.tile([P, 1], f32)
        nc.vector.memset(neg_ln_t, -math.log(TEMP))

        # Preload the Ln/Exp/Square ACT table set before any input lands, so
        # the first real ACT op (the w squared-norms gating the AllGather)
        # doesn't pay the ~2.7us lazy table load.
        preload = small.tile([P, 1], f32)
        nc.scalar.activation(out=preload, in_=neg_ln_t,
                             func=mybir.ActivationFunctionType.Exp)

        # PE observes the gpsimd (identity) semaphore here, so later real
        # transposes carry a single sync wait (Matmult allows only one).
        PSW = GW // 2 if half_psum else GW
        warm = psum.tile([P, PSW], f32, tag="S")
        nc.tensor.transpose(warm[:, :P], identity, identity)

        u4 = big.tile([P, M_CORE], f32)
        wown4 = big.tile([P, M_CORE], f32)
        wn = big.tile([P, M_CORE], f32)     # normalized own w shard
        u_t = big.tile([P, M_CORE], fp8)
        wtx = big.tile([P, GW], fp8)        # own w_t (d-major), staged for AG

        nsqu = small.tile([P, S_BLOCKS], f32)
        nsqw = small.tile([P, S_BLOCKS], f32)
        draw = small.tile([P, S_BLOCKS], f32)
        ru = small.tile([P, S_BLOCKS], f32)
        ruA = small.tile([P, S_BLOCKS], f32)
        lnt = small.tile([P, S_BLOCKS], f32)
        rw = small.tile([P, S_BLOCKS], f32)
        tacc = small.tile([P, S_BLOCKS * n_groups * 2], f32)
        nc.vector.memset(tacc, 0.0)
        parts = small.tile([P, S_BLOCKS * 6], f32)
        nc.vector.memset(parts, 0.0)
        ttl = small.tile([P, S_BLOCKS], f32)
        escr = big.tile([P, GW], bf16)      # TTR main output (discarded)

        cc_in = dram.tile([P, GW], fp8)
        cc_out = dram.tile([N_CORES * P, GW], fp8, addr_space="Shared")

        dma = nc.sync

        # ---------------- w (own shard) prep: normalize, transpose, AG ----
        dma.dma_start(out=wown4, in_=wown_ap)
        dma.dma_start(out=u4, in_=u_ap)
        Square = mybir.ActivationFunctionType.Square
        for s in range(S_BLOCKS):
            blk = slice(s * D, (s + 1) * D)
            sq = sqp.tile([P, D], f32, tag="sq")
            nc.scalar.activation(out=sq, in_=wown4[:, blk], func=Square,
                                 accum_out=nsqw[:, s:s + 1])
        # rw = 1/|w| = exp(-0.5*ln(nsqw))
        nc.scalar.activation(out=lnt, in_=nsqw, func=Ln)
        nc.scalar.activation(out=rw, in_=lnt, func=Exp, scale=-0.5)
        for s in range(S_BLOCKS):
            blk = slice(s * D, (s + 1) * D)
            nc.vector.tensor_scalar(out=wn[:, blk], in0=wown4[:, blk],
                                    scalar1=rw[:, s:s + 1], scalar2=None,
                                    op0=mult)
        # u transposes go first on PE (they only need u4); the wn transposes
        # follow as soon as wn is scaled. DVE drains wtx (gating the AG)
        # before u_t (gating the first matmul).
        psts = []
        for h in range(GW // PSW):
            pst = psum.tile([P, PSW], f32, tag="S")
            for s in range(PSW // D):
                so = h * (PSW // D) + s
                nc.tensor.transpose(pst[:, s * D:(s + 1) * D],
                                    u4[:, so * D:(so + 1) * D], identity)
            psts.append(pst)
        pws = []
        for h in range(GW // PSW):
            pw = psum.tile([P, PSW], f32, tag="S")
            for s in range(PSW // D):
                so = h * (PSW // D) + s
                nc.tensor.transpose(pw[:, s * D:(s + 1) * D],
                                    wn[:, so * D:(so + 1) * D], identity)
            pws.append(pw)
        for h, pw in enumerate(pws):
            nc.vector.tensor_copy(out=wtx[:, h * PSW:(h + 1) * PSW], in_=pw)
        dma.dma_start(out=cc_in, in_=wtx)
        nc.gpsimd.collective_compute(
            "AllGather", mybir.AluOpType.bypass,
            replica_groups=[list(range(N_CORES))],
            ins=[cc_in[:].opt()], outs=[cc_out[:].opt()])

        # ---------------- u prep (overlaps with AG) -----------------------
        for s in range(S_BLOCKS):
            blk = slice(s * D, (s + 1) * D)
            sq = sqp.tile([P, D], f32, tag="sq")
            nc.scalar.activation(out=sq, in_=u4[:, blk], func=Square,
                                 accum_out=nsqu[:, s:s + 1])
        for h, pst in enumerate(psts):
            nc.vector.tensor_copy(out=u_t[:, h * PSW:(h + 1) * PSW], in_=pst)
        # ru = 1/(T*|u|) = exp(-0.5*ln(nsqu) - ln(T))
        nc.scalar.activation(out=lnt, in_=nsqu, func=Ln)
        nc.scalar.activation(out=ru, in_=lnt, func=Exp, scale=-0.5, bias=neg_ln_t)
        nc.vector.tensor_scalar(out=ruA, in0=ru, scalar1=A_EXP, scalar2=None,
                                op0=mult)
        for s in range(S_BLOCKS):
            blk = slice(s * D, (s + 1) * D)
            sq = sqp.tile([P, D], f32, tag="sq")
            nc.vector.tensor_mul(out=sq, in0=u4[:, blk], in1=wown4[:, blk])
            nc.vector.reduce_sum(out=draw[:, s:s + 1], in_=sq,
                                 axis=mybir.AxisListType.X)
        dma.dma_start(out=nsqu_out[:], in_=nsqu)
        dma.dma_start(out=nsqw_out[:], in_=nsqw)
        dma.dma_start(out=draw_out[:], in_=draw)

        # ---------------- main loop --------------------------------------
        # Rotated group order: group t covers shard (pid+t) mod 8, so group 0
        # is this core's own shard -- its w_t (wtx) is already in SBUF and
        # carries no AllGather dependency. This hides the AG's latency (and
        # the inter-core NEFF start skew it absorbs) behind real compute.
        pid = dma.partition_id()
        dve_pending: dict = {}
        for g in range(n_groups):
            if g == 0:
                wt = wtx
            else:
                wt = wtp.tile([P, GW], fp8, tag="wt")
                blk_idx = (pid | g) - (pid & g)
                dma.dma_start(out=wt, in_=cc_out[bass.ds(blk_idx * P, P), :])
            # Group 0 (own shard) runs entirely on ACT: it is the only
            # AllGather-independent work, so stretching it on the single
            # fastest-exp engine maximizes how much of the collective's
            # latency gets hidden. In the last group, DVE m-blocks go first
            # so their pair-sum/reduce tail hides under the ACT tiles.
            ms = list(range(S_BLOCKS))
            if g == n_groups - 1:
                # Alternate DVE/ACT tiles with DVE biased early: DVE's
                # pair-sum tail then drains under ACT's remaining tiles
                # without ever starving ACT mid-group.
                dv = [m for m in ms if m in DVE_MS]
                av = [m for m in ms if m not in DVE_MS]
                ms = []
                while dv or av:
                    if dv:
                        ms.append(dv.pop(0))
                    if av:
                        ms.append(av.pop(0))
            seng = nc.gpsimd if gp_stt else nc.vector
            for m in ms:
                dve_tile = (m in DVE_MS) and (g > 0 or not own_act)
                e = None
                for h in range(GW // PSW):
                    ps = psum.tile([P, PSW], f32, tag="S")
                    for k4 in range(PSW // MM_NL):
                        nsl = slice(k4 * MM_NL, (k4 + 1) * MM_NL)
                        wsl = slice(h * PSW + k4 * MM_NL,
                                    h * PSW + (k4 + 1) * MM_NL)
                        nc.tensor.matmul(ps[:, nsl], u_t[:, m * D:(m + 1) * D],
                                         wt[:, wsl], start=True, stop=True)
                    if dve_tile:
                        if e is None:
                            e = ep.tile([P, GW], i16, tag=f"e{m}")
                        nc.vector.tensor_scalar(out=e[:, h * PSW:(h + 1) * PSW],
                                                in0=ps,
                                                scalar1=ruA[:, m:m + 1],
                                                scalar2=B_EXP, op0=mult, op1=add)
                    else:
                        es = esp.tile([P, PSW], bf16, tag="es")
                        if m in DVE_MS:
                            acc = parts[:, m * 6 + h:m * 6 + h + 1]
                        else:
                            acc = tacc[:, (m * n_groups + g) * 2 + h:
                                       (m * n_groups + g) * 2 + h + 1]
                        nc.scalar.activation(
                            out=es, in_=ps, func=Exp, scale=ru[:, m:m + 1],
                            accum_out=acc)
                        if act_memset:
                            nc.vector.memset(ps[:, :1], 0.0)
                if dve_tile:
                    store_gs = (1, 3, 5) if own_act else (0, 2, 4, 6)
                    if g in store_gs:
                        dve_pending[m] = e
                    elif not own_act or g in (2, 4, 6):
                        ea = dve_pending.pop(m)
                        slot = m * 6 + (1 + g // 2 if own_act else g // 2)
                        seng.scalar_tensor_tensor(
                            out=escr, in0=ea[:].bitcast(bf16), scalar=1.0,
                            in1=e[:].bitcast(bf16), op0=mult, op1=add,
                            accum_out=parts[:, slot:slot + 1])
                    else:  # own_act g == 7: odd tile out, fold alone
                        nc.vector.reduce_sum(out=parts[:, m * 6 + 5:m * 6 + 6],
                                             in_=e[:].bitcast(bf16),
                                             axis=mybir.AxisListType.X)
                if g == n_groups - 1:
                    if m in DVE_MS:
                        nc.vector.reduce_sum(out=ttl[:, m:m + 1],
                                             in_=parts[:, m * 6:(m + 1) * 6],
                                             axis=mybir.AxisListType.X)
                    else:
                        nc.vector.reduce_sum(
                            out=ttl[:, m:m + 1],
                            in_=tacc[:, m * n_groups * 2:(m + 1) * n_groups * 2],
                            axis=mybir.AxisListType.X)

        dma.dma_start(out=ttl_out[:], in_=ttl)

    nc.compile()
    return nc


_NC_CACHE: dict = {}


def _get_nc() -> bass.Bass:
    if "nc" not in _NC_CACHE:
        _NC_CACHE["nc"] = build_kernel()
    return _NC_CACHE["nc"]


def make_in_maps(emb: np.ndarray) -> list[dict]:
    in_maps = []
    for c in range(N_CORES):
        sl = slice(c * M_CORE, (c + 1) * M_CORE)
        in_maps.append({
            "u": np.ascontiguousarray(emb[0, sl]),
            "wown": np.ascontiguousarray(emb[1, sl]),
        })
    return in_maps


def _combine(results: list[dict], temp: float = TEMP) -> np.float32:
    total = 0.0
    for r in results:
        ttl = r["ttl"].astype(np.float64).reshape(-1)
        draw = r["draw"].astype(np.float64).reshape(-1)
        nsqu = r["nsqu"].astype(np.float64).reshape(-1)
        nsqw = r["nsqw"].astype(np.float64).reshape(-1)
        norms = np.maximum(np.sqrt(nsqu), 1e-12) * np.maximum(np.sqrt(nsqw), 1e-12)
        total += np.sum(np.log(ttl) - draw / (temp * norms))
    return np.float32(total)


def kernel(emb: np.ndarray) -> np.ndarray:
    from concourse.bass_utils import run_bass_kernel_spmd

    emb = np.ascontiguousarray(np.asarray(emb, dtype=np.float32))
    assert emb.shape == (2, N_TOTAL, D), emb.shape
    nc = _get_nc()
    res = run_bass_kernel_spmd(nc, make_in_maps(emb), core_ids=list(range(N_CORES)))
    return np.array(_combine(res.results), dtype=np.float32)


# revision 41
# speedup vs baseline: 1.6504x; 1.0197x over previous
"""Contrastive (InfoNCE-style) loss kernel for Trainium2, SPMD over 8 NeuronCores.

Math: emb [2, N, D] -> v1 = l2norm(emb[0]), v2 = l2norm(emb[1])
  loss = -sum_i [ (v1_i . v2_i)/T - log sum_j exp((v1_i . v2_j)/T) ]

Sharding: each core receives only its 2048-row shard of BOTH views (2MB),
so the host ships 16MB total instead of 80MB. The second view is
normalized, transposed (d-major) and fp8e4-cast on its owner core, then
all-gathered on-device (512KB/rank) - w-prep is done once globally
instead of 8x per core.

Groups are processed in rotated order: group t covers shard (pid+t) mod 8
via runtime-offset DMA (bass.ds on the partition id), so group 0 is the
core's own shard, whose w_t is already in SBUF with no AllGather
dependency. That hides most of the collective's ~70us latency (and the
inter-core NEFF start skew it absorbs) behind real compute.

Each core computes its [2048, 16384] similarity block in 128x1024 PSUM
half-tiles (fp8 matmul, fp32 accumulate; 4 PSUM buffers give the PE a
deep enough pipeline window to stream at its instruction-rate floor).
The exp+row-sum over the tiles is split across two engines running
concurrently:
  - ScalarE (ACT): exact exp with fused row-sum (accum_out), u's
    normalization folded into the per-partition activation scale. The
    u/w squared-norms also run here (Square is in the same ACT table
    set as Ln/Exp, so there are no table switches).
  - VectorE (DVE): Schraudolph-style exp - one tensor_scalar computes
    a*x+b and converts to int16, whose bits ARE the bf16 encoding of
    2^(x*log2e) (piecewise-linear mantissa); pairs of tiles are then
    summed with a fused row-reduce (scalar_tensor_tensor accum_out).
    The constant b is bias-fitted so the sawtooth error averages out
    over the 16k-term row sums; the log() in the loss compresses what
    remains below 1e-4.

Per-row partials (ttl, draw, |u|^2, |w|^2) return to the host, which
combines in f64.
"""

import math
from contextlib import ExitStack

import numpy as np

import concourse.bass as bass
import concourse.bacc as bacc
import concourse.mybir as mybir
from concourse.masks import make_identity
from concourse.tile import TileContext

P = 128
D = 128
TEMP = 0.2
N_TOTAL = 16384
N_CORES = 8
M_CORE = N_TOTAL // N_CORES   # 2048 rows per core
S_BLOCKS = M_CORE // P        # 16 row-blocks per core
GW = M_CORE                   # sim columns per group (= one shard)
MM_N = 512                    # moving-operand columns per matmul

# Schraudolph exp constants: int16(round(A*x + B)) bit-viewed as bf16
# approximates e^x.  B includes a bias fit (c=0.058) that zeroes the mean
# relative error over the actual similarity distribution.
A_EXP = 128.0 * 1.4426950408889634
B_EXP = 16256.0 - 128.0 * 0.058

# m-blocks whose exp runs on DVE (Schraudolph); the rest run on ACT.
DVE_MS = (1, 4, 6, 9, 11, 14)

f32 = mybir.dt.float32
bf16 = mybir.dt.bfloat16
fp8 = mybir.dt.float8e4
i16 = mybir.dt.int16


def build_kernel(dve_ms=DVE_MS, mm_n: int = MM_N,
                 act_memset: bool = False, half_psum: bool = True,
                 gp_stt: bool = False, own_act: bool = True) -> bass.Bass:
    DVE_MS = tuple(dve_ms)
    MM_NL = mm_n
    n_groups = N_CORES
    mult = mybir.AluOpType.mult
    add = mybir.AluOpType.add
    Ln = mybir.ActivationFunctionType.Ln
    Exp = mybir.ActivationFunctionType.Exp

    nc = bacc.Bacc(num_devices=N_CORES)
    u_in = nc.declare_dram_parameter("u", [M_CORE, D], f32, isOutput=False)
    wown_in = nc.declare_dram_parameter("wown", [M_CORE, D], f32, isOutput=False)
    ttl_out = nc.declare_dram_parameter("ttl", [P, S_BLOCKS], f32, isOutput=True)
    draw_out = nc.declare_dram_parameter("draw", [P, S_BLOCKS], f32, isOutput=True)
    nsqu_out = nc.declare_dram_parameter("nsqu", [P, S_BLOCKS], f32, isOutput=True)
    nsqw_out = nc.declare_dram_parameter("nsqw", [P, S_BLOCKS], f32, isOutput=True)

    # Partition p holds a contiguous slab of rows: u4[p, s*D+d] = u[p*S_BLOCKS+s, d]
    u_ap = u_in[:].rearrange("(p s) d -> p (s d)", p=P)
    wown_ap = wown_in[:].rearrange("(p s) d -> p (s d)", p=P)

    with TileContext(nc) as tc, ExitStack() as ctx:
        consts = ctx.enter_context(tc.tile_pool(name="consts", bufs=1))
        big = ctx.enter_context(tc.tile_pool(name="big", bufs=1))
        small = ctx.enter_context(tc.tile_pool(name="small", bufs=1))
        sqp = ctx.enter_context(tc.tile_pool(name="sqp", bufs=2))
        esp = ctx.enter_context(tc.tile_pool(name="esp", bufs=2))
        ep = ctx.enter_context(tc.tile_pool(name="ep", bufs=2))
        wtp = ctx.enter_context(tc.tile_pool(name="wtp", bufs=3))
        psum = ctx.enter_context(tc.tile_pool(
            name="psum", bufs=4 if half_psum else 2, space="PSUM"))
        dram = ctx.enter_context(tc.tile_pool(name="dram", bufs=1, space="DRAM"))

        identity = consts.tile([P, P], f32)
        make_identity(nc, identity)
        neg_ln_t = consts.tile([P, 1], f32)
        nc.vector.memset(neg_ln_t, -math.log(TEMP))

        # Preload the Ln/Exp/Square ACT table set before any input lands, so
        # the first real ACT op (the w squared-norms gating the AllGather)
        # doesn't pay the ~2.7us lazy table load.
        preload = small.tile([P, 1], f32)
        nc.scalar.activation(out=preload, in_=neg_ln_t,
                             func=mybir.ActivationFunctionType.Exp)

        # PE observes the gpsimd (identity) semaphore here, so later real
        # transposes carry a single sync wait (Matmult allows only one).
        PSW = GW // 2 if half_psum else GW
        warm = psum.tile([P, PSW], f32, tag="S")
        nc.tensor.transpose(warm[:, :P], identity, identity)

        u4 = big.tile([P, M_CORE], f32)
        wown4 = big.tile([P, M_CORE], f32)
        wn = big.tile([P, M_CORE], f32)     # normalized own w shard
        u_t = big.tile([P, M_CORE], fp8)
        wtx = big.tile([P, GW], fp8)        # own w_t (d-major), staged for AG

        nsqu = small.tile([P, S_BLOCKS], f32)
        nsqw = small.tile([P, S_BLOCKS], f32)
        draw = small.tile([P, S_BLOCKS], f32)
        ru = small.tile([P, S_BLOCKS], f32)
        ruA = small.tile([P, S_BLOCKS], f32)
        lnt = small.tile([P, S_BLOCKS], f32)
        rw = small.tile([P, S_BLOCKS], f32)
        tacc = small.tile([P, S_BLOCKS * n_groups * 2], f32)
        nc.vector.memset(tacc, 0.0)
        parts = small.tile([P, S_BLOCKS * 6], f32)
        nc.vector.memset(parts, 0.0)
        ttl = small.tile([P, S_BLOCKS], f32)
        escr = big.tile([P, GW], bf16)      # TTR main output (discarded)

        cc_in = dram.tile([P, GW], fp8)
        cc_out = dram.tile([N_CORES * P, GW], fp8, addr_space="Shared")

        dma = nc.sync

        # ---------------- w (own shard) prep: normalize, transpose, AG ----
        dma.dma_start(out=wown4, in_=wown_ap)
        dma.dma_start(out=u4, in_=u_ap)
        Square = mybir.ActivationFunctionType.Square
        for s in range(S_BLOCKS):
            blk = slice(s * D, (s + 1) * D)
            sq = sqp.tile([P, D], f32, tag="sq")
            nc.scalar.activation(out=sq, in_=wown4[:, blk], func=Square,
                                 accum_out=nsqw[:, s:s + 1])
        # rw = 1/|w| = exp(-0.5*ln(nsqw))
        nc.scalar.activation(out=lnt, in_=nsqw, func=Ln)
        nc.scalar.activation(out=rw, in_=lnt, func=Exp, scale=-0.5)
        for s in range(S_BLOCKS):
            blk = slice(s * D, (s + 1) * D)
            nc.vector.tensor_scalar(out=wn[:, blk], in0=wown4[:, blk],
                                    scalar1=rw[:, s:s + 1], scalar2=None,
                                    op0=mult)
        # u transposes go first on PE (they only need u4); the wn transposes
        # follow as soon as wn is scaled. DVE drains wtx (gating the AG)
        # before u_t (gating the first matmul).
        psts = []
        for h in range(GW // PSW):
            pst = psum.tile([P, PSW], f32, tag="S")
            for s in range(PSW // D):
                so = h * (PSW // D) + s
                nc.tensor.transpose(pst[:, s * D:(s + 1) * D],
                                    u4[:, so * D:(so + 1) * D], identity)
            psts.append(pst)
        pws = []
        for h in range(GW // PSW):
            pw = psum.tile([P, PSW], f32, tag="S")
            for s in range(PSW // D):
                so = h * (PSW // D) + s
                nc.tensor.transpose(pw[:, s * D:(s + 1) * D],
                                    wn[:, so * D:(so + 1) * D], identity)
            pws.append(pw)
        for h, pw in enumerate(pws):
            nc.vector.tensor_copy(out=wtx[:, h * PSW:(h + 1) * PSW], in_=pw)
        dma.dma_start(out=cc_in, in_=wtx)
        nc.gpsimd.collective_compute(
            "AllGather", mybir.AluOpType.bypass,
            replica_groups=[list(range(N_CORES))],
            ins=[cc_in[:].opt()], outs=[cc_out[:].opt()])

        # ---------------- u prep (overlaps with AG) -----------------------
        for s in range(S_BLOCKS):
            blk = slice(s * D, (s + 1) * D)
            sq = sqp.tile([P, D], f32, tag="sq")
            nc.scalar.activation(out=sq, in_=u4[:, blk], func=Square,
                                 accum_out=nsqu[:, s:s + 1])
        for h, pst in enumerate(psts):
            nc.vector.tensor_copy(out=u_t[:, h * PSW:(h + 1) * PSW], in_=pst)
        # ru = 1/(T*|u|) = exp(-0.5*ln(nsqu) - ln(T))
        nc.scalar.activation(out=lnt, in_=nsqu, func=Ln)
        nc.scalar.activation(out=ru, in_=lnt, func=Exp, scale=-0.5, bias=neg_ln_t)
        nc.vector.tensor_scalar(out=ruA, in0=ru, scalar1=A_EXP, scalar2=None,
                                op0=mult)
        for s in range(S_BLOCKS):
            blk = slice(s * D, (s + 1) * D)
            sq = sqp.tile([P, D], f32, tag="sq")
            nc.vector.tensor_mul(out=sq, in0=u4[:, blk], in1=wown4[:, blk])
            nc.vector.reduce_sum(out=draw[:, s:s + 1], in_=sq,
                                 axis=mybir.AxisListType.X)
        dma.dma_start(out=nsqu_out[:], in_=nsqu)
        dma.dma_start(out=nsqw_out[:], in_=nsqw)
        dma.dma_start(out=draw_out[:], in_=draw)

        # ---------------- main loop --------------------------------------
        # Rotated group order: group t covers shard (pid+t) mod 8, so group 0
        # is this core's own shard -- its w_t (wtx) is already in SBUF and
        # carries no AllGather dependency. This hides the AG's latency (and
        # the inter-core NEFF start skew it absorbs) behind real compute.
        pid = dma.partition_id()
        dve_pending: dict = {}
        for g in range(n_groups):
            if g == 0:
                wt = wtx
            else:
                wt = wtp.tile([P, GW], fp8, tag="wt")
                blk_idx = (pid | g) - (pid & g)
                dma.dma_start(out=wt, in_=cc_out[bass.ds(blk_idx * P, P), :])
            # Group 0 (own shard) runs entirely on ACT: it is the only
            # AllGather-independent work, so stretching it on the single
            # fastest-exp engine maximizes how much of the collective's
            # latency gets hidden. In the last group, DVE m-blocks go first
            # so their pair-sum/reduce tail hides under the ACT tiles.
            ms = list(range(S_BLOCKS))
            if g == n_groups - 1:
                # Alternate DVE/ACT tiles with DVE biased early: DVE's
                # pair-sum tail then drains under ACT's remaining tiles
                # without ever starving ACT mid-group.
                dv = [m for m in ms if m in DVE_MS]
                av = [m for m in ms if m not in DVE_MS]
                ms = []
                while dv or av:
                    if dv:
                        ms.append(dv.pop(0))
                    if av:
                        ms.append(av.pop(0))
            seng = nc.gpsimd if gp_stt else nc.vector
            for m in ms:
                dve_tile = (m in DVE_MS) and (g > 0 or not own_act)
                e = None
                for h in range(GW // PSW):
                    ps = psum.tile([P, PSW], f32, tag="S")
                    for k4 in range(PSW // MM_NL):
                        nsl = slice(k4 * MM_NL, (k4 + 1) * MM_NL)
                        wsl = slice(h * PSW + k4 * MM_NL,
                                    h * PSW + (k4 + 1) * MM_NL)
                        nc.tensor.matmul(ps[:, nsl], u_t[:, m * D:(m + 1) * D],
                                         wt[:, wsl], start=True, stop=True)
                    if dve_tile:
                        if e is None:
                            e = ep.tile([P, GW], i16, tag=f"e{m}")
                        nc.vector.tensor_scalar(out=e[:, h * PSW:(h + 1) * PSW],
                                                in0=ps,
                                                scalar1=ruA[:, m:m + 1],
                                                scalar2=B_EXP, op0=mult, op1=add)
                    else:
                        es = esp.tile([P, PSW], bf16, tag="es")
                        if m in DVE_MS:
                            acc = parts[:, m * 6 + h:m * 6 + h + 1]
                        else:
                            acc = tacc[:, (m * n_groups + g) * 2 + h:
                                       (m * n_groups + g) * 2 + h + 1]
                        nc.scalar.activation(
                            out=es, in_=ps, func=Exp, scale=ru[:, m:m + 1],
                            accum_out=acc)
                        if act_memset:
                            nc.vector.memset(ps[:, :1], 0.0)
                if dve_tile:
                    if not own_act:
                        if g in (0, 2, 4, 6):
                            dve_pending[m] = e
                        else:
                            ea = dve_pending.pop(m)
                            slot = m * 6 + g // 2
                            seng.scalar_tensor_tensor(
                                out=escr, in0=ea[:].bitcast(bf16), scalar=1.0,
                                in1=e[:].bitcast(bf16), op0=mult, op1=add,
                                accum_out=parts[:, slot:slot + 1])
                    elif g in (1, 3, 5):
                        dve_pending[m] = e
                    elif g in (2, 4):
                        # (g1,g2) and (g3,g4): fused pair-sum + row-reduce
                        ea = dve_pending.pop(m)
                        slot = m * 6 + 1 + g // 2
                        seng.scalar_tensor_tensor(
                            out=escr, in0=ea[:].bitcast(bf16), scalar=1.0,
                            in1=e[:].bitcast(bf16), op0=mult, op1=add,
                            accum_out=parts[:, slot:slot + 1])
                    elif g == 6:
                        # (g5,g6): plain bf16 add runs in DVE 2x mode; the
                        # row-reduce folds into g7's STT below.
                        ea = dve_pending.pop(m)
                        pt = ep.tile([P, GW], i16, tag=f"p{m}", bufs=1)
                        nc.vector.tensor_add(out=pt[:].bitcast(bf16),
                                             in0=ea[:].bitcast(bf16),
                                             in1=e[:].bitcast(bf16))
                        dve_pending[m] = pt
                    else:  # g == 7: (g5+g6) + g7 with fused row-reduce
                        ea = dve_pending.pop(m)
                        seng.scalar_tensor_tensor(
                            out=escr, in0=ea[:].bitcast(bf16), scalar=1.0,
                            in1=e[:].bitcast(bf16), op0=mult, op1=add,
                            accum_out=parts[:, m * 6 + 4:m * 6 + 5])
                if g == n_groups - 1:
                    if m in DVE_MS:
                        nc.vector.reduce_sum(out=ttl[:, m:m + 1],
                                             in_=parts[:, m * 6:(m + 1) * 6],
                                             axis=mybir.AxisListType.X)
                    else:
                        nc.vector.reduce_sum(
                            out=ttl[:, m:m + 1],
                            in_=tacc[:, m * n_groups * 2:(m + 1) * n_groups * 2],
                            axis=mybir.AxisListType.X)

        dma.dma_start(out=ttl_out[:], in_=ttl)

    nc.compile()
    return nc


_NC_CACHE: dict = {}


def _get_nc() -> bass.Bass:
    if "nc" not in _NC_CACHE:
        _NC_CACHE["nc"] = build_kernel()
    return _NC_CACHE["nc"]


def make_in_maps(emb: np.ndarray) -> list[dict]:
    in_maps = []
    for c in range(N_CORES):
        sl = slice(c * M_CORE, (c + 1) * M_CORE)
        in_maps.append({
            "u": np.ascontiguousarray(emb[0, sl]),
            "wown": np.ascontiguousarray(emb[1, sl]),
        })
    return in_maps


def _combine(results: list[dict], temp: float = TEMP) -> np.float32:
    total = 0.0
    for r in results:
        ttl = r["ttl"].astype(np.float64).reshape(-1)
        draw = r["draw"].astype(np.float64).reshape(-1)
        nsqu = r["nsqu"].astype(np.float64).reshape(-1)
        nsqw = r["nsqw"].astype(np.float64).reshape(-1)
        norms = np.maximum(np.sqrt(nsqu), 1e-12) * np.maximum(np.sqrt(nsqw), 1e-12)
        total += np.sum(np.log(ttl) - draw / (temp * norms))
    return np.float32(total)


def kernel(emb: np.ndarray) -> np.ndarray:
    from concourse.bass_utils import run_bass_kernel_spmd

    emb = np.ascontiguousarray(np.asarray(emb, dtype=np.float32))
    assert emb.shape == (2, N_TOTAL, D), emb.shape
    nc = _get_nc()
    res = run_bass_kernel_spmd(nc, make_in_maps(emb), core_ids=list(range(N_CORES)))
    return np.array(_combine(res.results), dtype=np.float32)


# revision 43
# speedup vs baseline: 1.7156x; 1.0395x over previous
"""Contrastive (InfoNCE-style) loss kernel for Trainium2, SPMD over 8 NeuronCores.

Math: emb [2, N, D] -> v1 = l2norm(emb[0]), v2 = l2norm(emb[1])
  loss = -sum_i [ (v1_i . v2_i)/T - log sum_j exp((v1_i . v2_j)/T) ]

Sharding: each core receives only its 2048-row shard of BOTH views (2MB),
so the host ships 16MB total instead of 80MB. The second view is
normalized, transposed (d-major) and fp8e4-cast on its owner core, then
all-gathered on-device (512KB/rank) - w-prep is done once globally
instead of 8x per core.

Groups are processed in rotated order: group t covers shard (pid+t) mod 8
via runtime-offset DMA (bass.ds on the partition id), so group 0 is the
core's own shard, whose w_t is already in SBUF with no AllGather
dependency. That hides most of the collective's ~70us latency (and the
inter-core NEFF start skew it absorbs) behind real compute.

Each core computes its [2048, 16384] similarity block in 128x1024 PSUM
half-tiles (fp8 matmul, fp32 accumulate; 4 PSUM buffers give the PE a
deep enough pipeline window to stream at its instruction-rate floor).
The exp+row-sum over the tiles is split across two engines running
concurrently:
  - ScalarE (ACT): exact exp with fused row-sum (accum_out), u's
    normalization folded into the per-partition activation scale. The
    u/w squared-norms also run here (Square is in the same ACT table
    set as Ln/Exp, so there are no table switches).
  - VectorE (DVE): Schraudolph-style exp - one tensor_scalar computes
    a*x+b and converts to int16, whose bits ARE the bf16 encoding of
    2^(x*log2e) (piecewise-linear mantissa); pairs of tiles are then
    summed with a fused row-reduce (scalar_tensor_tensor accum_out).
    The constant b is bias-fitted so the sawtooth error averages out
    over the 16k-term row sums; the log() in the loss compresses what
    remains below 1e-4.

Per-row partials (ttl, draw, |u|^2, |w|^2) return to the host, which
combines in f64.
"""

import math
from contextlib import ExitStack

import numpy as np

import concourse.bass as bass
import concourse.bacc as bacc
import concourse.mybir as mybir
from concourse.masks import make_identity
from concourse.tile import TileContext

P = 128
D = 128
TEMP = 0.2
N_TOTAL = 16384
N_CORES = 8
M_CORE = N_TOTAL // N_CORES   # 2048 rows per core
S_BLOCKS = M_CORE // P        # 16 row-blocks per core
GW = M_CORE                   # sim columns per group (= one shard)
MM_N = 512                    # moving-operand columns per matmul

# Schraudolph exp constants: int16(round(A*x + B)) bit-viewed as bf16
# approximates e^x.  B includes a bias fit (c=0.058) that zeroes the mean
# relative error over the actual similarity distribution.
A_EXP = 128.0 * 1.4426950408889634
B_EXP = 16256.0 - 128.0 * 0.058

# m-blocks whose exp runs on DVE (Schraudolph); the rest run on ACT.
DVE_MS = (1, 4, 6, 9, 11, 14)

f32 = mybir.dt.float32
bf16 = mybir.dt.bfloat16
fp8 = mybir.dt.float8e4
i16 = mybir.dt.int16


def build_kernel(dve_ms=DVE_MS, mm_n: int = MM_N,
                 act_memset: bool = False, half_psum: bool = True,
                 gp_stt: bool = False, own_act: bool = True) -> bass.Bass:
    DVE_MS = tuple(dve_ms)
    MM_NL = mm_n
    n_groups = N_CORES
    mult = mybir.AluOpType.mult
    add = mybir.AluOpType.add
    Ln = mybir.ActivationFunctionType.Ln
    Exp = mybir.ActivationFunctionType.Exp

    nc = bacc.Bacc(num_devices=N_CORES)
    u_in = nc.declare_dram_parameter("u", [M_CORE, D], f32, isOutput=False)
    wown_in = nc.declare_dram_parameter("wown", [M_CORE, D], f32, isOutput=False)
    ttl_out = nc.declare_dram_parameter("ttl", [P, S_BLOCKS], f32, isOutput=True)
    draw_out = nc.declare_dram_parameter("draw", [P, S_BLOCKS], f32, isOutput=True)
    nsqu_out = nc.declare_dram_parameter("nsqu", [P, S_BLOCKS], f32, isOutput=True)
    nsqw_out = nc.declare_dram_parameter("nsqw", [P, S_BLOCKS], f32, isOutput=True)

    # Partition p holds a contiguous slab of rows: u4[p, s*D+d] = u[p*S_BLOCKS+s, d]
    u_ap = u_in[:].rearrange("(p s) d -> p (s d)", p=P)
    wown_ap = wown_in[:].rearrange("(p s) d -> p (s d)", p=P)

    with TileContext(nc) as tc, ExitStack() as ctx:
        consts = ctx.enter_context(tc.tile_pool(name="consts", bufs=1))
        big = ctx.enter_context(tc.tile_pool(name="big", bufs=1))
        small = ctx.enter_context(tc.tile_pool(name="small", bufs=1))
        sqp = ctx.enter_context(tc.tile_pool(name="sqp", bufs=2))
        esp = ctx.enter_context(tc.tile_pool(name="esp", bufs=2))
        ep = ctx.enter_context(tc.tile_pool(name="ep", bufs=2))
        wtp = ctx.enter_context(tc.tile_pool(name="wtp", bufs=3))
        psum = ctx.enter_context(tc.tile_pool(
            name="psum", bufs=4 if half_psum else 2, space="PSUM"))
        dram = ctx.enter_context(tc.tile_pool(name="dram", bufs=1, space="DRAM"))

        identity = consts.tile([P, P], f32)
        make_identity(nc, identity)
        neg_ln_t = consts.tile([P, 1], f32)
        nc.vector.memset(neg_ln_t, -math.log(TEMP))

        # Preload the Ln/Exp/Square ACT table set before any input lands, so
        # the first real ACT op (the w squared-norms gating the AllGather)
        # doesn't pay the ~2.7us lazy table load.
        preload = small.tile([P, 1], f32)
        nc.scalar.activation(out=preload, in_=neg_ln_t,
                             func=mybir.ActivationFunctionType.Exp)

        # PE observes the gpsimd (identity) semaphore here, so later real
        # transposes carry a single sync wait (Matmult allows only one).
        PSW = GW // 2 if half_psum else GW
        warm = psum.tile([P, PSW], f32, tag="S")
        nc.tensor.transpose(warm[:, :P], identity, identity)

        u4 = big.tile([P, M_CORE], f32)
        wown4 = big.tile([P, M_CORE], f32)
        wn = big.tile([P, M_CORE], f32)     # normalized own w shard
        u_t = big.tile([P, M_CORE], fp8)
        wtx = big.tile([P, GW], fp8)        # own w_t (d-major), staged for AG

        nsqu = small.tile([P, S_BLOCKS], f32)
        nsqw = small.tile([P, S_BLOCKS], f32)
        draw = small.tile([P, S_BLOCKS], f32)
        ru = small.tile([P, S_BLOCKS], f32)
        ruA = small.tile([P, S_BLOCKS], f32)
        lnt = small.tile([P, S_BLOCKS], f32)
        rw = small.tile([P, S_BLOCKS], f32)
        tacc = small.tile([P, S_BLOCKS * n_groups * 2], f32)
        nc.vector.memset(tacc, 0.0)
        parts = small.tile([P, S_BLOCKS * 6], f32)
        nc.vector.memset(parts, 0.0)
        ttl = small.tile([P, S_BLOCKS], f32)
        escr = big.tile([P, GW], bf16)      # TTR main output (discarded)

        cc_in = dram.tile([P, GW], fp8)
        cc_out = dram.tile([N_CORES * P, GW], fp8, addr_space="Shared")

        dma = nc.sync

        # ---------------- w (own shard) prep: normalize, transpose, AG ----
        # Inputs stream in quarters so the squares/Ln/Exp/scale chain that
        # gates the AllGather trigger starts ~4us after the first quarter
        # lands instead of waiting for the full 1MB.
        QD = M_CORE // 4
        for q in range(4):
            dma.dma_start(out=wown4[:, q * QD:(q + 1) * QD],
                          in_=wown_ap[:, q * QD:(q + 1) * QD])
        for q in range(4):
            dma.dma_start(out=u4[:, q * QD:(q + 1) * QD],
                          in_=u_ap[:, q * QD:(q + 1) * QD])
        Square = mybir.ActivationFunctionType.Square
        for s in range(S_BLOCKS):
            blk = slice(s * D, (s + 1) * D)
            sq = sqp.tile([P, D], f32, tag="sq")
            nc.scalar.activation(out=sq, in_=wown4[:, blk], func=Square,
                                 accum_out=nsqw[:, s:s + 1])
            # rw = 1/|w| = exp(-0.5*ln(nsqw)), in half-batches so the wn
            # scale and transpose pipeline per-half behind the squares.
            if s in (S_BLOCKS // 2 - 1, S_BLOCKS - 1):
                hs = slice(0, S_BLOCKS // 2) if s < S_BLOCKS - 1 else \
                    slice(S_BLOCKS // 2, S_BLOCKS)
                nc.scalar.activation(out=lnt[:, hs], in_=nsqw[:, hs], func=Ln)
                nc.scalar.activation(out=rw[:, hs], in_=lnt[:, hs], func=Exp,
                                     scale=-0.5)
        for s in range(S_BLOCKS):
            blk = slice(s * D, (s + 1) * D)
            nc.vector.tensor_scalar(out=wn[:, blk], in0=wown4[:, blk],
                                    scalar1=rw[:, s:s + 1], scalar2=None,
                                    op0=mult)
        # wn transposes go first on PE (they gate the AllGather trigger);
        # the u transposes follow (u_t is only needed by the first matmul).
        # DVE likewise drains wtx before u_t.
        pws = []
        for h in range(GW // PSW):
            pw = psum.tile([P, PSW], f32, tag="S")
            for s in range(PSW // D):
                so = h * (PSW // D) + s
                nc.tensor.transpose(pw[:, s * D:(s + 1) * D],
                                    wn[:, so * D:(so + 1) * D], identity)
            pws.append(pw)
            nc.vector.tensor_copy(out=wtx[:, h * PSW:(h + 1) * PSW], in_=pw)
        psts = []
        for h in range(GW // PSW):
            pst = psum.tile([P, PSW], f32, tag="S")
            for s in range(PSW // D):
                so = h * (PSW // D) + s
                nc.tensor.transpose(pst[:, s * D:(s + 1) * D],
                                    u4[:, so * D:(so + 1) * D], identity)
            psts.append(pst)
        dma.dma_start(out=cc_in, in_=wtx)
        nc.gpsimd.collective_compute(
            "AllGather", mybir.AluOpType.bypass,
            replica_groups=[list(range(N_CORES))],
            ins=[cc_in[:].opt()], outs=[cc_out[:].opt()])

        # ---------------- u prep (overlaps with AG) -----------------------
        for s in range(S_BLOCKS):
            blk = slice(s * D, (s + 1) * D)
            sq = sqp.tile([P, D], f32, tag="sq")
            nc.scalar.activation(out=sq, in_=u4[:, blk], func=Square,
                                 accum_out=nsqu[:, s:s + 1])
        for h, pst in enumerate(psts):
            nc.vector.tensor_copy(out=u_t[:, h * PSW:(h + 1) * PSW], in_=pst)
        # ru = 1/(T*|u|) = exp(-0.5*ln(nsqu) - ln(T))
        nc.scalar.activation(out=lnt, in_=nsqu, func=Ln)
        nc.scalar.activation(out=ru, in_=lnt, func=Exp, scale=-0.5, bias=neg_ln_t)
        nc.vector.tensor_scalar(out=ruA, in0=ru, scalar1=A_EXP, scalar2=None,
                                op0=mult)
        for s in range(S_BLOCKS):
            blk = slice(s * D, (s + 1) * D)
            sq = sqp.tile([P, D], f32, tag="sq")
            nc.vector.tensor_mul(out=sq, in0=u4[:, blk], in1=wown4[:, blk])
            nc.vector.reduce_sum(out=draw[:, s:s + 1], in_=sq,
                                 axis=mybir.AxisListType.X)
        dma.dma_start(out=nsqu_out[:], in_=nsqu)
        dma.dma_start(out=nsqw_out[:], in_=nsqw)
        dma.dma_start(out=draw_out[:], in_=draw)

        # ---------------- main loop --------------------------------------
        # Rotated group order: group t covers shard (pid+t) mod 8, so group 0
        # is this core's own shard -- its w_t (wtx) is already in SBUF and
        # carries no AllGather dependency. This hides the AG's latency (and
        # the inter-core NEFF start skew it absorbs) behind real compute.
        pid = dma.partition_id()
        dve_pending: dict = {}
        for g in range(n_groups):
            if g == 0:
                wt = wtx
            else:
                wt = wtp.tile([P, GW], fp8, tag="wt")
                blk_idx = (pid | g) - (pid & g)
                dma.dma_start(out=wt, in_=cc_out[bass.ds(blk_idx * P, P), :])
            # Group 0 (own shard) runs entirely on ACT: it is the only
            # AllGather-independent work, so stretching it on the single
            # fastest-exp engine maximizes how much of the collective's
            # latency gets hidden. In the last group, DVE m-blocks go first
            # so their pair-sum/reduce tail hides under the ACT tiles.
            ms = list(range(S_BLOCKS))
            if g == n_groups - 1:
                # Alternate DVE/ACT tiles with DVE biased early: DVE's
                # pair-sum tail then drains under ACT's remaining tiles
                # without ever starving ACT mid-group.
                dv = [m for m in ms if m in DVE_MS]
                av = [m for m in ms if m not in DVE_MS]
                ms = []
                while dv or av:
                    if dv:
                        ms.append(dv.pop(0))
                    if av:
                        ms.append(av.pop(0))
            seng = nc.gpsimd if gp_stt else nc.vector
            for m in ms:
                dve_tile = (m in DVE_MS) and (g > 0 or not own_act)
                e = None
                for h in range(GW // PSW):
                    ps = psum.tile([P, PSW], f32, tag="S")
                    for k4 in range(PSW // MM_NL):
                        nsl = slice(k4 * MM_NL, (k4 + 1) * MM_NL)
                        wsl = slice(h * PSW + k4 * MM_NL,
                                    h * PSW + (k4 + 1) * MM_NL)
                        nc.tensor.matmul(ps[:, nsl], u_t[:, m * D:(m + 1) * D],
                                         wt[:, wsl], start=True, stop=True)
                    if dve_tile:
                        if e is None:
                            e = ep.tile([P, GW], i16, tag=f"e{m}")
                        nc.vector.tensor_scalar(out=e[:, h * PSW:(h + 1) * PSW],
                                                in0=ps,
                                                scalar1=ruA[:, m:m + 1],
                                                scalar2=B_EXP, op0=mult, op1=add)
                    else:
                        es = esp.tile([P, PSW], bf16, tag="es")
                        if m in DVE_MS:
                            acc = parts[:, m * 6 + h:m * 6 + h + 1]
                        else:
                            acc = tacc[:, (m * n_groups + g) * 2 + h:
                                       (m * n_groups + g) * 2 + h + 1]
                        nc.scalar.activation(
                            out=es, in_=ps, func=Exp, scale=ru[:, m:m + 1],
                            accum_out=acc)
                        if act_memset:
                            nc.vector.memset(ps[:, :1], 0.0)
                if dve_tile:
                    if not own_act:
                        if g in (0, 2, 4, 6):
                            dve_pending[m] = e
                        else:
                            ea = dve_pending.pop(m)
                            slot = m * 6 + g // 2
                            seng.scalar_tensor_tensor(
                                out=escr, in0=ea[:].bitcast(bf16), scalar=1.0,
                                in1=e[:].bitcast(bf16), op0=mult, op1=add,
                                accum_out=parts[:, slot:slot + 1])
                    elif g in (1, 3, 5):
                        dve_pending[m] = e
                    elif g in (2, 4):
                        # (g1,g2) and (g3,g4): fused pair-sum + row-reduce
                        ea = dve_pending.pop(m)
                        slot = m * 6 + 1 + g // 2
                        seng.scalar_tensor_tensor(
                            out=escr, in0=ea[:].bitcast(bf16), scalar=1.0,
                            in1=e[:].bitcast(bf16), op0=mult, op1=add,
                            accum_out=parts[:, slot:slot + 1])
                    elif g == 6:
                        # (g5,g6): plain bf16 add runs in DVE 2x mode; the
                        # row-reduce folds into g7's STT below.
                        ea = dve_pending.pop(m)
                        pt = ep.tile([P, GW], i16, tag=f"p{m}", bufs=1)
                        nc.vector.tensor_add(out=pt[:].bitcast(bf16),
                                             in0=ea[:].bitcast(bf16),
                                             in1=e[:].bitcast(bf16))
                        dve_pending[m] = pt
                    else:  # g == 7: (g5+g6) + g7 with fused row-reduce
                        ea = dve_pending.pop(m)
                        seng.scalar_tensor_tensor(
                            out=escr, in0=ea[:].bitcast(bf16), scalar=1.0,
                            in1=e[:].bitcast(bf16), op0=mult, op1=add,
                            accum_out=parts[:, m * 6 + 4:m * 6 + 5])
                if g == n_groups - 1:
                    if m in DVE_MS:
                        nc.vector.reduce_sum(out=ttl[:, m:m + 1],
                                             in_=parts[:, m * 6:(m + 1) * 6],
                                             axis=mybir.AxisListType.X)
                    else:
                        nc.vector.reduce_sum(
                            out=ttl[:, m:m + 1],
                            in_=tacc[:, m * n_groups * 2:(m + 1) * n_groups * 2],
                            axis=mybir.AxisListType.X)

        dma.dma_start(out=ttl_out[:], in_=ttl)

    nc.compile()
    return nc


_NC_CACHE: dict = {}


def _get_nc() -> bass.Bass:
    if "nc" not in _NC_CACHE:
        _NC_CACHE["nc"] = build_kernel()
    return _NC_CACHE["nc"]


def make_in_maps(emb: np.ndarray) -> list[dict]:
    in_maps = []
    for c in range(N_CORES):
        sl = slice(c * M_CORE, (c + 1) * M_CORE)
        in_maps.append({
            "u": np.ascontiguousarray(emb[0, sl]),
            "wown": np.ascontiguousarray(emb[1, sl]),
        })
    return in_maps


def _combine(results: list[dict], temp: float = TEMP) -> np.float32:
    total = 0.0
    for r in results:
        ttl = r["ttl"].astype(np.float64).reshape(-1)
        draw = r["draw"].astype(np.float64).reshape(-1)
        nsqu = r["nsqu"].astype(np.float64).reshape(-1)
        nsqw = r["nsqw"].astype(np.float64).reshape(-1)
        norms = np.maximum(np.sqrt(nsqu), 1e-12) * np.maximum(np.sqrt(nsqw), 1e-12)
        total += np.sum(np.log(ttl) - draw / (temp * norms))
    return np.float32(total)


def kernel(emb: np.ndarray) -> np.ndarray:
    from concourse.bass_utils import run_bass_kernel_spmd

    emb = np.ascontiguousarray(np.asarray(emb, dtype=np.float32))
    assert emb.shape == (2, N_TOTAL, D), emb.shape
    nc = _get_nc()
    res = run_bass_kernel_spmd(nc, make_in_maps(emb), core_ids=list(range(N_CORES)))
    return np.array(_combine(res.results), dtype=np.float32)
